# revision 17
# baseline (speedup 1.0000x reference)
"""MultiHeadAttention Trainium2 kernel (8 NeuronCores, SPMD).

Reference computation (B=4, T=1024, D=768, H=12, Dh=64):
    q = x @ Wq.T ; k = x @ Wk.T ; v = x @ Wv.T       (per-head reshape)
    attn = softmax((q @ k.T) / 8)
    out = (attn @ v) @ Wo.T + bo

Sharding: 8 cores = 4 batches x 2 head-halves (6 heads each). Each core
computes a [1024, 768] partial of the output projection for its 6 heads;
the host sums the two partials per batch and adds the bias.

Per-core dataflow (all matmuls fp32 data, fp32r PE mode):
    xT [768,1024] (host-pretransposed) -> SBUF
    qT,kT = (W x)  in [384,1024] layout; v in [1024,384] layout
    S.T tiles [kt=128, qt=512] = kT_head.T @ qT_head   (K=64 contraction)
    expS = exp(S.T) via ScalarE reading PSUM
    ctx.T psum [128, qt] = [v_head | ones].T @ expS    (K=kt accumulate)
        rows 0:64  = unnormalized ctx.T, rows 64:128 = softmax denominator
        (ones-block makes the denominator broadcast free)
    ctxT_norm = ctx.T * 1/denom   (DVE)
    out_partial = ctxT_norm.T @ Wo_slice.T             (K=384 accumulate)
"""

import numpy as np

import concourse.bass as bass
import concourse.mybir as mybir
from concourse import bacc
from concourse.tile import TileContext
from concourse.bass_utils import run_bass_kernel_spmd

FP = mybir.dt.float32
FPR = mybir.dt.float32r
AF = mybir.ActivationFunctionType

B, T, D = 4, 1024, 768
H, DH = 12, 64
NCORES = 8
HPC = 6           # heads per core
DPC = HPC * DH    # 384 head-dims per core
KC = D // 128     # 6 contraction chunks for d_in
MC = DPC // 128   # 3 chunks of per-core head dims
NT = T // 512     # 2 free-dim tiles of tokens
TT = T // 128     # 8 partition tiles of tokens


def emit_mha(tc, xT, wq, wk, wv, wo, ones, out, ctx):
    nc = tc.nc

    singles = ctx.enter_context(tc.tile_pool(name="singles", bufs=1))
    proj_psum = ctx.enter_context(tc.tile_pool(name="proj_psum", bufs=2, space="PSUM"))
    scores_psum = ctx.enter_context(
        tc.tile_pool(name="scores_psum", bufs=2, space="PSUM")
    )
    ctx_psum = ctx.enter_context(tc.tile_pool(name="ctx_psum", bufs=2, space="PSUM"))
    expS_pool = ctx.enter_context(tc.tile_pool(name="expS", bufs=8))
    rcp_pool = ctx.enter_context(tc.tile_pool(name="rcp", bufs=2))
    out_pool = ctx.enter_context(tc.tile_pool(name="outsb", bufs=3))

    # ---------------- load weights + x ----------------
    wq_sb = singles.tile([128, KC, DPC], FPR, name="wq_sb", tag="wq_sb")
    wk_sb = singles.tile([128, KC, DPC], FPR, name="wk_sb", tag="wk_sb")
    wv_sb = singles.tile([128, KC, DPC], FPR, name="wv_sb", tag="wv_sb")
    wo_sb = singles.tile([128, MC, D], FPR, name="wo_sb", tag="wo_sb")
    xT_sb = singles.tile([128, KC, T], FPR, name="xT_sb", tag="xT_sb")
    nc.sync.dma_start(out=wq_sb, in_=wq.rearrange("(c p) d -> p c d", p=128).bitcast(FPR))
    nc.sync.dma_start(out=wk_sb, in_=wk.rearrange("(c p) d -> p c d", p=128).bitcast(FPR))
    nc.sync.dma_start(out=wv_sb, in_=wv.rearrange("(c p) d -> p c d", p=128).bitcast(FPR))
    nc.sync.dma_start(out=wo_sb, in_=wo.rearrange("(c p) d -> p c d", p=128).bitcast(FPR))
    nc.sync.dma_start(out=xT_sb, in_=xT.rearrange("(c p) t -> p c t", p=128).bitcast(FPR))

    qT_sb = singles.tile([128, MC, T], FPR, name="qT_sb", tag="qT_sb")
    kT_sb = singles.tile([128, MC, T], FPR, name="kT_sb", tag="kT_sb")
    ctxT_sb = singles.tile([128, MC, T], FPR, name="ctxT_sb", tag="ctxT_sb")

    # v tiles [t_tile, 6 heads x (64 v cols + 1 ones col)]: the ones column
    # makes each head's ctx matmul also produce its softmax denominator
    # (psum row 64) in the same stream. memset can't write fp32r, so the
    # ones come from a tiny DRAM input.
    v_sb = []
    for i in range(TT):
        vt = singles.tile([128, HPC, DH + 1], FPR, name=f"v_sb{i}", tag=f"v_sb{i}")
        nc.sync.dma_start(
            out=vt[:, :, DH : DH + 1], in_=ones.bitcast(FPR)
        )
        v_sb.append(vt)

    # ---------------- QKV projections ----------------
    # qT/kT: out[m=dout_chunk(128), n=t(512)] = sum_c w[c,m].T @ xT[c,n]
    for m in range(MC):
        for w_sb, dst in ((wq_sb, qT_sb), (wk_sb, kT_sb)):
            for n in range(NT):
                ps = proj_psum.tile([128, 512], FP, name="ps_qk", tag="proj")
                for c in range(KC):
                    nc.tensor.matmul(
                        ps,
                        lhsT=w_sb[:, c, m * 128 : (m + 1) * 128],
                        rhs=xT_sb[:, c, n * 512 : (n + 1) * 512],
                        start=(c == 0),
                        stop=(c == KC - 1),
                    )
                nc.vector.tensor_copy(dst[:, m, n * 512 : (n + 1) * 512], ps)

    # v: out[m=t_tile(128), n=dh(384)] = sum_c xT[c,m].T @ wv[c,n]
    for mt in range(TT):
        ps = proj_psum.tile([128, DPC], FP, name="ps_v", tag="proj")
        for c in range(KC):
            nc.tensor.matmul(
                ps,
                lhsT=xT_sb[:, c, mt * 128 : (mt + 1) * 128],
                rhs=wv_sb[:, c, :],
                start=(c == 0),
                stop=(c == KC - 1),
            )
        nc.vector.tensor_copy(v_sb[mt][:, :, 0:DH], ps)

    # ---------------- attention, head pairs ----------------
    # head h lives at partition 64*(h%2), chunk h//2 of qT/kT/ctxT
    for hp in range(MC):
        for qt in range(NT):
            for h in (2 * hp, 2 * hp + 1):
                po = 64 * (h % 2)
                ch = h // 2
                kT_h = kT_sb[po : po + 64, ch, :]
                qT_h = qT_sb[po : po + 64, ch, qt * 512 : (qt + 1) * 512]

                exps = []
                for g in range(4):
                    ps = scores_psum.tile([128, 1024], FP, name="ps_s", tag="scores")
                    for r2 in range(2):
                        j = 2 * g + r2
                        nc.tensor.matmul(
                            ps[:, r2 * 512 : (r2 + 1) * 512],
                            lhsT=kT_h[:, j * 128 : (j + 1) * 128],
                            rhs=qT_h,
                            start=True,
                            stop=True,
                        )
                    ex = expS_pool.tile([128, 1024], FPR, name="ex", tag="expS")
                    nc.scalar.activation(ex, ps, AF.Exp)
                    exps.append(ex)

                # ctx.T accumulate over kt: lhsT = [v_head | ones] (M=65):
                # psum rows 0:64 = unnormalized ctx.T, row 64 = denominator
                pc = ctx_psum.tile([128, 512], FP, name="pc", tag="ctx")
                for j in range(TT):
                    ex_j = exps[j // 2][:, (j % 2) * 512 : (j % 2 + 1) * 512]
                    nc.tensor.matmul(
                        pc[0:65, :],
                        lhsT=v_sb[j][:, h, :],
                        rhs=ex_j,
                        start=(j == 0),
                        stop=(j == TT - 1),
                    )

                # 1/denom on row 64, then broadcast it over 64 partitions
                # with an address-mode (partition-step-0) SBUF->SBUF DMA
                rcp = rcp_pool.tile([1, 512], FP, name="rcp", tag="rcp")
                nc.vector.reciprocal(rcp, pc[64:65, :])
                rcpb = rcp_pool.tile([64, 512], FP, name="rcpb", tag="rcpb")
                nc.gpsimd.partition_broadcast(rcpb, rcp, channels=64)
                nc.vector.tensor_mul(
                    ctxT_sb[po : po + 64, ch, qt * 512 : (qt + 1) * 512],
                    pc[0:64, :],
                    rcpb,
                )

    # ---------------- output projection ----------------
    # out[m=t_tile(128), n=dout(384)] = sum_c ctxT[c,m].T @ wo[c,n]
    for mt in range(TT):
        osb = out_pool.tile([128, D], FP, name="osb", tag="outsb")
        for n2 in range(2):
            ps = proj_psum.tile([128, 384], FP, name="ps_o", tag="proj")
            for c in range(MC):
                nc.tensor.matmul(
                    ps,
                    lhsT=ctxT_sb[:, c, mt * 128 : (mt + 1) * 128],
                    rhs=wo_sb[:, c, n2 * 384 : (n2 + 1) * 384],
                    start=(c == 0),
                    stop=(c == MC - 1),
                )
            nc.vector.tensor_copy(osb[:, n2 * 384 : (n2 + 1) * 384], ps)
        nc.sync.dma_start(out=out[mt * 128 : (mt + 1) * 128, :], in_=osb)


_PROGRAM = None


def build_program():
    global _PROGRAM
    if _PROGRAM is not None:
        return _PROGRAM
    nc = bacc.Bacc("TRN2", target_bir_lowering=False, debug=False, num_devices=NCORES)
    xT = nc.dram_tensor("xT", (D, T), FP, kind="ExternalInput").ap()
    wq = nc.dram_tensor("wq", (D, DPC), FP, kind="ExternalInput").ap()
    wk = nc.dram_tensor("wk", (D, DPC), FP, kind="ExternalInput").ap()
    wv = nc.dram_tensor("wv", (D, DPC), FP, kind="ExternalInput").ap()
    wo = nc.dram_tensor("wo", (DPC, D), FP, kind="ExternalInput").ap()
    ones = nc.dram_tensor("ones", (128, HPC), FP, kind="ExternalInput").ap()
    out = nc.dram_tensor("out", (T, D), FP, kind="ExternalOutput").ap()
    from contextlib import ExitStack

    with TileContext(nc) as tc, ExitStack() as st:
        emit_mha(tc, xT, wq, wk, wv, wo, ones, out, st)
    nc.compile()
    _PROGRAM = nc
    return nc


def make_in_maps(x, Wq, Wk, Wv, Wo):
    x = np.asarray(x, dtype=np.float32)
    in_maps = []
    xTs = [np.ascontiguousarray(x[b].T) for b in range(B)]
    for core in range(NCORES):
        b, hh = core // 2, core % 2
        sl = slice(hh * DPC, (hh + 1) * DPC)
        in_maps.append(
            {
                "xT": xTs[b],
                "wq": np.ascontiguousarray((np.asarray(Wq)[sl] * 0.125).T, np.float32),
                "wk": np.ascontiguousarray(np.asarray(Wk)[sl].T, np.float32),
                "wv": np.ascontiguousarray(np.asarray(Wv)[sl].T, np.float32),
                "wo": np.ascontiguousarray(np.asarray(Wo)[:, sl].T, np.float32),
                "ones": np.ones((128, HPC), np.float32),
            }
        )
    return in_maps


def kernel(x, Wq, Wk, Wv, Wo, bo):
    nc = build_program()
    in_maps = make_in_maps(x, Wq, Wk, Wv, Wo)
    res = run_bass_kernel_spmd(nc, in_maps, core_ids=list(range(NCORES)))
    bo = np.asarray(bo, dtype=np.float32)
    out = np.empty((B, T, D), dtype=np.float32)
    for b in range(B):
        out[b] = res.results[2 * b]["out"] + res.results[2 * b + 1]["out"] + bo
    return out


# revision 20
# speedup vs baseline: 1.2552x; 1.2552x over previous
"""MultiHeadAttention Trainium2 kernel (8 NeuronCores, SPMD).

Reference computation (B=4, T=1024, D=768, H=12, Dh=64):
    q = x @ Wq.T ; k = x @ Wk.T ; v = x @ Wv.T       (per-head reshape)
    attn = softmax((q @ k.T) / 8)
    out = (attn @ v) @ Wo.T + bo

Sharding: 8 cores = 4 batches x 2 head-halves (6 heads each). Each core
computes a [1024, 768] partial of the output projection for its 6 heads;
the host sums the two partials per batch and adds the bias.

Per-core dataflow (all matmuls fp32 data, fp32r PE mode):
    xT [768,1024] (host-pretransposed) -> SBUF
    qT,kT = (W x)  in [384,1024] layout; v in [1024,384] layout
    S.T tiles [kt=128, qt=512] = kT_head.T @ qT_head   (K=64 contraction)
    expS = exp(S.T) via ScalarE reading PSUM
    ctx.T psum [128, qt] = [v_head | ones].T @ expS    (K=kt accumulate)
        rows 0:64  = unnormalized ctx.T, rows 64:128 = softmax denominator
        (ones-block makes the denominator broadcast free)
    ctxT_norm = ctx.T * 1/denom   (DVE)
    out_partial = ctxT_norm.T @ Wo_slice.T             (K=384 accumulate)
"""

import numpy as np

import concourse.bass as bass
import concourse.mybir as mybir
from concourse import bacc
from concourse.tile import TileContext
from concourse.bass_utils import run_bass_kernel_spmd

FP = mybir.dt.float32
FPR = mybir.dt.float32r
AF = mybir.ActivationFunctionType

B, T, D = 4, 1024, 768
H, DH = 12, 64
NCORES = 8
HPC = 6           # heads per core
DPC = HPC * DH    # 384 head-dims per core
KC = D // 128     # 6 contraction chunks for d_in
MC = DPC // 128   # 3 chunks of per-core head dims
NT = T // 512     # 2 free-dim tiles of tokens
TT = T // 128     # 8 partition tiles of tokens


def emit_mha(tc, xT, wq, wk, wv, wo, ones, out, ctx):
    nc = tc.nc

    singles = ctx.enter_context(tc.tile_pool(name="singles", bufs=1))
    proj_psum = ctx.enter_context(tc.tile_pool(name="proj_psum", bufs=2, space="PSUM"))
    scores_psum = ctx.enter_context(
        tc.tile_pool(name="scores_psum", bufs=2, space="PSUM")
    )
    ctx_psum = ctx.enter_context(tc.tile_pool(name="ctx_psum", bufs=2, space="PSUM"))
    expS_pool = ctx.enter_context(tc.tile_pool(name="expS", bufs=8))
    rcp_pool = ctx.enter_context(tc.tile_pool(name="rcp", bufs=4))
    out_pool = ctx.enter_context(tc.tile_pool(name="outsb", bufs=3))

    # ---------------- staged input DMAs ----------------
    # xT first (QKV-critical), chunked so matmul accumulation chases arrival;
    # wq/wk sliced per m-tile so head-pair hp unblocks after slice hp.
    xT_sb = singles.tile([128, KC, T], FPR, name="xT_sb", tag="xT_sb")
    xTr = xT.rearrange("(c p) t -> p c t", p=128).bitcast(FPR)
    wq_sb = singles.tile([128, KC, DPC], FPR, name="wq_sb", tag="wq_sb")
    wk_sb = singles.tile([128, KC, DPC], FPR, name="wk_sb", tag="wk_sb")
    wv_sb = singles.tile([128, KC, DPC], FPR, name="wv_sb", tag="wv_sb")
    wo_sb = singles.tile([128, MC, D], FPR, name="wo_sb", tag="wo_sb")
    wqr = wq.rearrange("(c p) d -> p c d", p=128).bitcast(FPR)
    wkr = wk.rearrange("(c p) d -> p c d", p=128).bitcast(FPR)
    # first q/k slice weights, then xT token-halves (n0 of every chunk first
    # so the first scores tile unblocks after half the xT bytes)
    nc.sync.dma_start(out=wk_sb[:, :, 0:128], in_=wkr[:, :, 0:128])
    nc.sync.dma_start(out=wq_sb[:, :, 0:128], in_=wqr[:, :, 0:128])
    for n in range(NT):
        for c in range(KC):
            nc.sync.dma_start(
                out=xT_sb[:, c, n * 512 : (n + 1) * 512],
                in_=xTr[:, c, n * 512 : (n + 1) * 512],
            )
    nc.sync.dma_start(out=wv_sb, in_=wv.rearrange("(c p) d -> p c d", p=128).bitcast(FPR))
    for m in range(1, MC):
        nc.sync.dma_start(out=wk_sb[:, :, m * 128 : (m + 1) * 128],
                          in_=wkr[:, :, m * 128 : (m + 1) * 128])
        nc.sync.dma_start(out=wq_sb[:, :, m * 128 : (m + 1) * 128],
                          in_=wqr[:, :, m * 128 : (m + 1) * 128])
    nc.sync.dma_start(out=wo_sb, in_=wo.rearrange("(c p) d -> p c d", p=128).bitcast(FPR))

    qT_sb = singles.tile([128, MC, T], FPR, name="qT_sb", tag="qT_sb")
    kT_sb = singles.tile([128, MC, T], FPR, name="kT_sb", tag="kT_sb")
    ctxT_sb = singles.tile([128, MC, T], FPR, name="ctxT_sb", tag="ctxT_sb")

    # v tiles [t_tile, 6 heads x (64 v cols + 1 ones col)]: the ones column
    # makes each head's ctx matmul also produce its softmax denominator
    # (psum row 64) in the same stream. memset can't write fp32r, so the
    # ones come from a tiny DRAM input.
    v_sb = []
    for i in range(TT):
        vt = singles.tile([128, HPC, DH + 1], FPR, name=f"v_sb{i}", tag=f"v_sb{i}")
        nc.sync.dma_start(out=vt[:, :, DH : DH + 1], in_=ones.bitcast(FPR))
        v_sb.append(vt)

    def qk_proj(m):
        # qT/kT chunk m: out[m=dout(128), n=t(512)] = sum_c w[c,m].T @ xT[c,n]
        for n in range(NT):
            for w_sb, dst in ((wk_sb, kT_sb), (wq_sb, qT_sb)):
                ps = proj_psum.tile([128, 512], FP, name="ps_qk", tag="proj")
                for c in range(KC):
                    nc.tensor.matmul(
                        ps,
                        lhsT=w_sb[:, c, m * 128 : (m + 1) * 128],
                        rhs=xT_sb[:, c, n * 512 : (n + 1) * 512],
                        start=(c == 0),
                        stop=(c == KC - 1),
                    )
                nc.vector.tensor_copy(dst[:, m, n * 512 : (n + 1) * 512], ps)

    def v_proj():
        # v: out[m=t_tile(128), n=dh(384)] = sum_c xT[c,m].T @ wv[c,n]
        for mt in range(TT):
            ps = proj_psum.tile([128, DPC], FP, name="ps_v", tag="proj")
            for c in range(KC):
                nc.tensor.matmul(
                    ps,
                    lhsT=xT_sb[:, c, mt * 128 : (mt + 1) * 128],
                    rhs=wv_sb[:, c, :],
                    start=(c == 0),
                    stop=(c == KC - 1),
                )
            nc.vector.tensor_copy(v_sb[mt][:, :, 0:DH], ps)

    def pair_scores(hp, qt):
        # heads 2hp (partitions 0:64) and 2hp+1 (partitions 64:128) of chunk
        # hp. Scores for the two heads are interleaved per-matmul: disjoint
        # PE row groups (tile_position rows 0 vs 64) can run concurrently.
        pair = []
        for h in (2 * hp, 2 * hp + 1):
            po = 64 * (h % 2)
            kT_h = kT_sb[po : po + 64, hp, :]
            qT_h = qT_sb[po : po + 64, hp, qt * 512 : (qt + 1) * 512]
            pair.append((h, po, kT_h, qT_h, []))

        for g in range(4):
            pss = [
                scores_psum.tile([128, 1024], FP, name="ps_s", tag="scores")
                for _ in pair
            ]
            for r2 in range(2):
                j = 2 * g + r2
                for (h, po, kT_h, qT_h, exps), ps in zip(pair, pss):
                    nc.tensor.matmul(
                        ps[:, r2 * 512 : (r2 + 1) * 512],
                        lhsT=kT_h[:, j * 128 : (j + 1) * 128],
                        rhs=qT_h,
                        start=True,
                        stop=True,
                    )
            for (h, po, kT_h, qT_h, exps), ps in zip(pair, pss):
                ex = expS_pool.tile([128, 1024], FPR, name="ex", tag="expS")
                nc.scalar.activation(ex, ps, AF.Exp)
                exps.append(ex)
        return pair

    def pair_ctx(hp, qt, pair):
        for h, po, kT_h, qT_h, exps in pair:
                # ctx.T accumulate over kt: lhsT = [v_head | ones] (M=65):
                # psum rows 0:64 = unnormalized ctx.T, row 64 = denominator
            pc = ctx_psum.tile([128, 512], FP, name="pc", tag="ctx")
            for j in range(TT):
                ex_j = exps[j // 2][:, (j % 2) * 512 : (j % 2 + 1) * 512]
                nc.tensor.matmul(
                    pc[0:65, :],
                    lhsT=v_sb[j][:, h, :],
                    rhs=ex_j,
                    start=(j == 0),
                    stop=(j == TT - 1),
                )
            # 1/denom to partition 0, broadcast over 64 partitions
            # (gpsimd custom ops require base-partition-0 operands)
            rcp = rcp_pool.tile([1, 512], FP, name="rcp", tag="rcp")
            nc.vector.reciprocal(rcp, pc[64:65, :])
            rcpb = rcp_pool.tile([64, 512], FP, name="rcpb", tag="rcpb")
            nc.gpsimd.partition_broadcast(rcpb, rcp, channels=64)
            nc.vector.tensor_mul(
                ctxT_sb[po : po + 64, hp, qt * 512 : (qt + 1) * 512],
                pc[0:64, :],
                rcpb,
            )

    def out_proj(mts):
        # out[m=t_tile(128), n=dout(384)] = sum_c ctxT[c,m].T @ wo[c,n]
        for mt in mts:
            osb = out_pool.tile([128, D], FP, name="osb", tag="outsb")
            for n2 in range(2):
                ps = proj_psum.tile([128, 384], FP, name="ps_o", tag="proj")
                for c in range(MC):
                    nc.tensor.matmul(
                        ps,
                        lhsT=ctxT_sb[:, c, mt * 128 : (mt + 1) * 128],
                        rhs=wo_sb[:, c, n2 * 384 : (n2 + 1) * 384],
                        start=(c == 0),
                        stop=(c == MC - 1),
                    )
                nc.vector.tensor_copy(osb[:, n2 * 384 : (n2 + 1) * 384], ps)
            nc.sync.dma_start(out=out[mt * 128 : (mt + 1) * 128, :], in_=osb)

    # interleaved schedule: scores of head pair 0 start as soon as q/k chunk
    # 0 lands (ACT warms up early); v projection overlaps those exps; the
    # output projection's t-halves chase the last head pair's two qt halves.
    qk_proj(0)
    p00 = pair_scores(0, 0)
    v_proj()
    pair_ctx(0, 0, p00)
    p01 = pair_scores(0, 1)
    pair_ctx(0, 1, p01)
    qk_proj(1)
    for qt in range(NT):
        pair_ctx(1, qt, pair_scores(1, qt))
    qk_proj(2)
    p20 = pair_scores(2, 0)
    pair_ctx(2, 0, p20)
    p21 = pair_scores(2, 1)
    out_proj(range(0, TT // 2))
    pair_ctx(2, 1, p21)
    out_proj(range(TT // 2, TT))


_PROGRAM = None


def build_program():
    global _PROGRAM
    if _PROGRAM is not None:
        return _PROGRAM
    nc = bacc.Bacc("TRN2", target_bir_lowering=False, debug=False, num_devices=NCORES)
    xT = nc.dram_tensor("xT", (D, T), FP, kind="ExternalInput").ap()
    wq = nc.dram_tensor("wq", (D, DPC), FP, kind="ExternalInput").ap()
    wk = nc.dram_tensor("wk", (D, DPC), FP, kind="ExternalInput").ap()
    wv = nc.dram_tensor("wv", (D, DPC), FP, kind="ExternalInput").ap()
    wo = nc.dram_tensor("wo", (DPC, D), FP, kind="ExternalInput").ap()
    ones = nc.dram_tensor("ones", (128, HPC), FP, kind="ExternalInput").ap()
    out = nc.dram_tensor("out", (T, D), FP, kind="ExternalOutput").ap()
    from contextlib import ExitStack

    with TileContext(nc) as tc, ExitStack() as st:
        emit_mha(tc, xT, wq, wk, wv, wo, ones, out, st)
    nc.compile()
    _PROGRAM = nc
    return nc


def make_in_maps(x, Wq, Wk, Wv, Wo):
    x = np.asarray(x, dtype=np.float32)
    in_maps = []
    xTs = [np.ascontiguousarray(x[b].T) for b in range(B)]
    for core in range(NCORES):
        b, hh = core // 2, core % 2
        sl = slice(hh * DPC, (hh + 1) * DPC)
        in_maps.append(
            {
                "xT": xTs[b],
                "wq": np.ascontiguousarray((np.asarray(Wq)[sl] * 0.125).T, np.float32),
                "wk": np.ascontiguousarray(np.asarray(Wk)[sl].T, np.float32),
                "wv": np.ascontiguousarray(np.asarray(Wv)[sl].T, np.float32),
                "wo": np.ascontiguousarray(np.asarray(Wo)[:, sl].T, np.float32),
                "ones": np.ones((128, HPC), np.float32),
            }
        )
    return in_maps


def kernel(x, Wq, Wk, Wv, Wo, bo):
    nc = build_program()
    in_maps = make_in_maps(x, Wq, Wk, Wv, Wo)
    res = run_bass_kernel_spmd(nc, in_maps, core_ids=list(range(NCORES)))
    bo = np.asarray(bo, dtype=np.float32)
    out = np.empty((B, T, D), dtype=np.float32)
    for b in range(B):
        out[b] = res.results[2 * b]["out"] + res.results[2 * b + 1]["out"] + bo
    return out


# revision 33
# speedup vs baseline: 1.2914x; 1.0289x over previous
"""MultiHeadAttention Trainium2 kernel (8 NeuronCores, SPMD).

Reference computation (B=4, T=1024, D=768, H=12, Dh=64):
    q = x @ Wq.T ; k = x @ Wk.T ; v = x @ Wv.T       (per-head reshape)
    attn = softmax((q @ k.T) / 8)
    out = (attn @ v) @ Wo.T + bo

Sharding: 8 cores = 4 batches x 2 head-halves (6 heads each). Each core
computes a [1024, 768] partial of the output projection for its 6 heads;
the host sums the two partials per batch and adds the bias.

Per-core dataflow (all matmuls fp32 data, fp32r PE mode):
    xT [768,1024] (host-pretransposed) -> SBUF
    qT,kT = (W x)  in [384,1024] layout; v in [1024,384] layout
    S.T tiles [kt=128, qt=512] = kT_head.T @ qT_head   (K=64 contraction)
    expS = exp(S.T) via ScalarE reading PSUM
    ctx.T psum [128, qt] = [v_head | ones].T @ expS    (K=kt accumulate)
        rows 0:64  = unnormalized ctx.T, rows 64:128 = softmax denominator
        (ones-block makes the denominator broadcast free)
    ctxT_norm = ctx.T * 1/denom   (DVE)
    out_partial = ctxT_norm.T @ Wo_slice.T             (K=384 accumulate)
"""

import numpy as np

import concourse.bass as bass
import concourse.mybir as mybir
from concourse import bacc
from concourse.tile import TileContext
from concourse.bass_utils import run_bass_kernel_spmd

FP = mybir.dt.float32
FPR = mybir.dt.float32r
AF = mybir.ActivationFunctionType

B, T, D = 4, 1024, 768
H, DH = 12, 64
NCORES = 8
HPC = 6           # heads per core
DPC = HPC * DH    # 384 head-dims per core
KC = D // 128     # 6 contraction chunks for d_in
MC = DPC // 128   # 3 chunks of per-core head dims
NT = T // 512     # 2 free-dim tiles of tokens
TT = T // 128     # 8 partition tiles of tokens


def emit_mha(tc, xT, wq, wk, wv, wo, ones, out, ctx):
    nc = tc.nc

    singles = ctx.enter_context(tc.tile_pool(name="singles", bufs=1))
    proj_psum = ctx.enter_context(tc.tile_pool(name="proj_psum", bufs=2, space="PSUM"))
    scores_psum = ctx.enter_context(
        tc.tile_pool(name="scores_psum", bufs=2, space="PSUM")
    )
    ctx_psum = ctx.enter_context(tc.tile_pool(name="ctx_psum", bufs=2, space="PSUM"))
    expS_pool = ctx.enter_context(tc.tile_pool(name="expS", bufs=8))
    rcp_pool = ctx.enter_context(tc.tile_pool(name="rcp", bufs=6))
    out_pool = ctx.enter_context(tc.tile_pool(name="outsb", bufs=8))

    # ---------------- staged input DMAs ----------------
    # xT first (QKV-critical), chunked so matmul accumulation chases arrival;
    # wq/wk sliced per m-tile so head-pair hp unblocks after slice hp.
    xT_sb = singles.tile([128, KC, T], FPR, name="xT_sb", tag="xT_sb")
    xTr = xT.rearrange("(c p) t -> p c t", p=128).bitcast(FPR)
    wq_sb = singles.tile([128, KC, DPC], FPR, name="wq_sb", tag="wq_sb")
    wk_sb = singles.tile([128, KC, DPC], FPR, name="wk_sb", tag="wk_sb")
    wv_sb = singles.tile([128, KC, DPC], FPR, name="wv_sb", tag="wv_sb")
    wo_sb = singles.tile([128, MC, D], FPR, name="wo_sb", tag="wo_sb")
    wqr = wq.rearrange("(c p) d -> p c d", p=128).bitcast(FPR)
    wkr = wk.rearrange("(c p) d -> p c d", p=128).bitcast(FPR)
    # first q/k slice weights, then xT token-halves (n0 of every chunk first
    # so the first scores tile unblocks after half the xT bytes)
    nc.sync.dma_start(out=wk_sb[:, :, 0:128], in_=wkr[:, :, 0:128])
    nc.sync.dma_start(out=wq_sb[:, :, 0:128], in_=wqr[:, :, 0:128])
    for n in range(NT):
        for c in range(KC):
            nc.sync.dma_start(
                out=xT_sb[:, c, n * 512 : (n + 1) * 512],
                in_=xTr[:, c, n * 512 : (n + 1) * 512],
            )
    nc.sync.dma_start(out=wv_sb, in_=wv.rearrange("(c p) d -> p c d", p=128).bitcast(FPR))
    for m in range(1, MC):
        nc.sync.dma_start(out=wk_sb[:, :, m * 128 : (m + 1) * 128],
                          in_=wkr[:, :, m * 128 : (m + 1) * 128])
        nc.sync.dma_start(out=wq_sb[:, :, m * 128 : (m + 1) * 128],
                          in_=wqr[:, :, m * 128 : (m + 1) * 128])
    nc.sync.dma_start(out=wo_sb, in_=wo.rearrange("(c p) d -> p c d", p=128).bitcast(FPR))

    qT_sb = singles.tile([128, MC, T], FPR, name="qT_sb", tag="qT_sb")
    kT_sb = singles.tile([128, MC, T], FPR, name="kT_sb", tag="kT_sb")
    ctxT_sb = singles.tile([128, MC, T], FPR, name="ctxT_sb", tag="ctxT_sb")

    # v tiles [t_tile, 6 heads x (64 v cols + 1 ones col)]: the ones column
    # makes each head's ctx matmul also produce its softmax denominator
    # (psum row 64) in the same stream. memset can't write fp32r, so the
    # ones come from a tiny DRAM input.
    v_sb = []
    for i in range(TT):
        vt = singles.tile([128, HPC, DH + 1], FPR, name=f"v_sb{i}", tag=f"v_sb{i}")
        nc.sync.dma_start(out=vt[:, :, DH : DH + 1], in_=ones.bitcast(FPR))
        v_sb.append(vt)

    def qk_proj(m):
        # qT/kT chunk m: out[m=dout(128), n=t(512)] = sum_c w[c,m].T @ xT[c,n]
        for n in range(NT):
            for w_sb, dst in ((wk_sb, kT_sb), (wq_sb, qT_sb)):
                ps = proj_psum.tile([128, 512], FP, name="ps_qk", tag="proj")
                for c in range(KC):
                    nc.tensor.matmul(
                        ps,
                        lhsT=w_sb[:, c, m * 128 : (m + 1) * 128],
                        rhs=xT_sb[:, c, n * 512 : (n + 1) * 512],
                        start=(c == 0),
                        stop=(c == KC - 1),
                    )
                nc.vector.tensor_copy(dst[:, m, n * 512 : (n + 1) * 512], ps)

    def v_proj():
        # v: out[m=t_tile(128), n=dh(384)] = sum_c xT[c,m].T @ wv[c,n]
        for mt in range(TT):
            ps = proj_psum.tile([128, DPC], FP, name="ps_v", tag="proj")
            for c in range(KC):
                nc.tensor.matmul(
                    ps,
                    lhsT=xT_sb[:, c, mt * 128 : (mt + 1) * 128],
                    rhs=wv_sb[:, c, :],
                    start=(c == 0),
                    stop=(c == KC - 1),
                )
            nc.vector.tensor_copy(v_sb[mt][:, :, 0:DH], ps)

    def pair_scores(hp, qt):
        # heads 2hp (partitions 0:64) and 2hp+1 (partitions 64:128) of chunk
        # hp. Scores for the two heads are interleaved per-matmul: disjoint
        # PE row groups (tile_position rows 0 vs 64) can run concurrently.
        pair = []
        for h in (2 * hp, 2 * hp + 1):
            po = 64 * (h % 2)
            kT_h = kT_sb[po : po + 64, hp, :]
            qT_h = qT_sb[po : po + 64, hp, qt * 512 : (qt + 1) * 512]
            pair.append((h, po, kT_h, qT_h, []))

        for g in range(4):
            pss = [
                scores_psum.tile([128, 1024], FP, name="ps_s", tag="scores")
                for _ in pair
            ]
            for r2 in range(2):
                j = 2 * g + r2
                for (h, po, kT_h, qT_h, exps), ps in zip(pair, pss):
                    nc.tensor.matmul(
                        ps[:, r2 * 512 : (r2 + 1) * 512],
                        lhsT=kT_h[:, j * 128 : (j + 1) * 128],
                        rhs=qT_h,
                        start=True,
                        stop=True,
                    )
            for (h, po, kT_h, qT_h, exps), ps in zip(pair, pss):
                ex = expS_pool.tile([128, 1024], FPR, name="ex", tag="expS")
                nc.scalar.activation(ex, ps, AF.Exp)
                exps.append(ex)
        return pair

    def pair_ctx(hp, qt, pair):
        for h, po, kT_h, qT_h, exps in pair:
                # ctx.T accumulate over kt: lhsT = [v_head | ones] (M=65):
                # psum rows 0:64 = unnormalized ctx.T, row 64 = denominator
            pc = ctx_psum.tile([128, 512], FP, name="pc", tag="ctx")
            for j in range(TT):
                ex_j = exps[j // 2][:, (j % 2) * 512 : (j % 2 + 1) * 512]
                nc.tensor.matmul(
                    pc[0:65, :],
                    lhsT=v_sb[j][:, h, :],
                    rhs=ex_j,
                    start=(j == 0),
                    stop=(j == TT - 1),
                )
            # 1/denom to partition 0, broadcast over 64 partitions
            # (gpsimd custom ops require base-partition-0 operands)
            rcp = rcp_pool.tile([1, 512], FP, name="rcp", tag="rcp")
            nc.vector.reciprocal(rcp, pc[64:65, :])
            rcpb = rcp_pool.tile([64, 512], FP, name="rcpb", tag="rcpb")
            nc.gpsimd.partition_broadcast(rcpb, rcp, channels=64)
            nc.vector.tensor_mul(
                ctxT_sb[po : po + 64, hp, qt * 512 : (qt + 1) * 512],
                pc[0:64, :],
                rcpb,
            )

    def out_proj(mts, pool=None, interleave=False):
        # out[m=t_tile(128), n=dout(384)] = sum_c ctxT[c,m].T @ wo[c,n]
        pool = pool or proj_psum
        if not interleave:
            for mt in mts:
                osb = out_pool.tile([128, D], FP, name="osb", tag="outsb")
                for n2 in range(2):
                    ps = pool.tile([128, 384], FP, name="ps_o", tag="proj")
                    for c in range(MC):
                        nc.tensor.matmul(
                            ps,
                            lhsT=ctxT_sb[:, c, mt * 128 : (mt + 1) * 128],
                            rhs=wo_sb[:, c, n2 * 384 : (n2 + 1) * 384],
                            start=(c == 0),
                            stop=(c == MC - 1),
                        )
                    if n2 == 0:
                        nc.vector.tensor_copy(osb[:, 0:384], ps)
                    else:
                        nc.scalar.copy(osb[:, 384:768], ps)
                nc.sync.dma_start(out=out[mt * 128 : (mt + 1) * 128, :], in_=osb)
            return
        # interleaved: emit chunk-0/1 matmuls for every (mt, n2) group first
        # (they only need ctxT chunks 0/1) so PE can fill stalls while the
        # last head pair's ctx normalize completes; chunk-2 + copies follow.
        groups = []
        for mt in mts:
            osb = out_pool.tile([128, D], FP, name="osb", tag="outsb")
            for n2 in range(2):
                ps = pool.tile([128, 384], FP, name="ps_t", tag="tail")
                groups.append((mt, n2, osb, ps))
        for c in range(MC):
            for mt, n2, osb, ps in groups:
                nc.tensor.matmul(
                    ps,
                    lhsT=ctxT_sb[:, c, mt * 128 : (mt + 1) * 128],
                    rhs=wo_sb[:, c, n2 * 384 : (n2 + 1) * 384],
                    start=(c == 0),
                    stop=(c == MC - 1),
                )
        done = set()
        for mt, n2, osb, ps in groups:
            nc.vector.tensor_copy(osb[:, n2 * 384 : (n2 + 1) * 384], ps)
            done.add((mt, n2))
            if (mt, 0) in done and (mt, 1) in done:
                nc.sync.dma_start(out=out[mt * 128 : (mt + 1) * 128, :], in_=osb)

    # interleaved schedule: scores of head pair 0 start as soon as q/k chunk
    # 0 lands (ACT warms up early); v projection overlaps those exps; the
    # output projection's t-halves chase the last head pair's two qt halves.
    qk_proj(0)
    p00 = pair_scores(0, 0)
    v_proj()
    pair_ctx(0, 0, p00)
    p01 = pair_scores(0, 1)
    pair_ctx(0, 1, p01)
    qk_proj(1)
    for qt in range(NT):
        pair_ctx(1, qt, pair_scores(1, qt))
    qk_proj(2)
    p20 = pair_scores(2, 0)
    pair_ctx(2, 0, p20)
    p21 = pair_scores(2, 1)
    out_proj(range(0, TT // 2))
    pair_ctx(2, 1, p21)
    out_proj(range(TT // 2, TT))


_PROGRAM = None


def build_program():
    global _PROGRAM
    if _PROGRAM is not None:
        return _PROGRAM
    nc = bacc.Bacc("TRN2", target_bir_lowering=False, debug=False, num_devices=NCORES)
    xT = nc.dram_tensor("xT", (D, T), FP, kind="ExternalInput").ap()
    wq = nc.dram_tensor("wq", (D, DPC), FP, kind="ExternalInput").ap()
    wk = nc.dram_tensor("wk", (D, DPC), FP, kind="ExternalInput").ap()
    wv = nc.dram_tensor("wv", (D, DPC), FP, kind="ExternalInput").ap()
    wo = nc.dram_tensor("wo", (DPC, D), FP, kind="ExternalInput").ap()
    ones = nc.dram_tensor("ones", (128, HPC), FP, kind="ExternalInput").ap()
    out = nc.dram_tensor("out", (T, D), FP, kind="ExternalOutput").ap()
    from contextlib import ExitStack

    with TileContext(nc) as tc, ExitStack() as st:
        emit_mha(tc, xT, wq, wk, wv, wo, ones, out, st)
    nc.compile()
    _PROGRAM = nc
    return nc


def make_in_maps(x, Wq, Wk, Wv, Wo):
    x = np.asarray(x, dtype=np.float32)
    in_maps = []
    xTs = [np.ascontiguousarray(x[b].T) for b in range(B)]
    for core in range(NCORES):
        b, hh = core // 2, core % 2
        sl = slice(hh * DPC, (hh + 1) * DPC)
        in_maps.append(
            {
                "xT": xTs[b],
                "wq": np.ascontiguousarray((np.asarray(Wq)[sl] * 0.125).T, np.float32),
                "wk": np.ascontiguousarray(np.asarray(Wk)[sl].T, np.float32),
                "wv": np.ascontiguousarray(np.asarray(Wv)[sl].T, np.float32),
                "wo": np.ascontiguousarray(np.asarray(Wo)[:, sl].T, np.float32),
                "ones": np.ones((128, HPC), np.float32),
            }
        )
    return in_maps


def kernel(x, Wq, Wk, Wv, Wo, bo):
    nc = build_program()
    in_maps = make_in_maps(x, Wq, Wk, Wv, Wo)
    res = run_bass_kernel_spmd(nc, in_maps, core_ids=list(range(NCORES)))
    bo = np.asarray(bo, dtype=np.float32)
    out = np.empty((B, T, D), dtype=np.float32)
    for b in range(B):
        out[b] = res.results[2 * b]["out"] + res.results[2 * b + 1]["out"] + bo
    return out
